# revision 102
# baseline (speedup 1.0000x reference)
"""Trainium2 Bass kernel for nn_FSE_Module_79147657331158.

Pipeline (per batch image, one per NeuronCore, 8-way data parallel):
  h1 = mish(BN1(conv3x3(x, w1)))          64 -> 128 ch
  h2 = mish(BN2(conv3x3(h1, w2))) + x     128 -> 64 ch
  cA, (cH,cV,cD) = haar_dwt2(h2)
  x_low  = cA
  x_high = mish(BNh(conv1x1(concat(cH,cV,cD), wh)))

v3 notes (372.8us TimelineSim vs the 621us v2 baseline; ~1.67x):
  - conv2 packs M=128: psum partitions = [row r x 64ch | row r+1 x 64ch].
    Inputs are single h1 rows r+d (d=-1..2); the weight block's upper half
    holds tap d-1.  12 matmuls @ ap=256 per row pair instead of 9 @ ap=512
    (PE cost is ap-proportional, K/M-independent) -> 2/3 the PE time.
  - h2 is stored [128p = ch x row-parity, 8 gi x 2 bb x 128 j], which the
    conv2 psum layout produces for free.  The fused DWT+convh contracts
    K=128 (ch x parity) with M=128 = [high-band | 0.5I (cA)]: 2 matmuls
    per output row-pair -- 4x fewer PE rows than v2's K=64/M=64 form.
    (High band sits in M 0-63: the custom DVE op below misbehaves at
    partition offset 64, verified empirically.)
  - mish is 3 passes: zb = psum+bias (ACT Identity or DVE TS -- frees the
    psum bank after ONE op, so 8 banks ride out the chain latency);
    u = exp(zb), m = (r2*u+r2)^2 = 0.5(1+u)^2 (ACT, one table set);
    then MISH_POST, a self-registered custom DVE op (8/8 ALU stages):
    bitwise-not reciprocal seed + one Newton step for y ~= 1/(m+0.5) =
    2/((1+u)^2+1), out = zb - y*zb = zb*tanh(softplus(zb)).  Seed/NR
    consts are minimax over D in [1,inf) pinned exact at D=1, killing
    the q->0 tail error (5.3e-3 max abs, ~1.2e-3 rel-fro per layer).
    The real act tables have no mish/softplus (and recip is in a
    different table set than exp -- 1.3us loads), hence this route.
  - GPSIMD/Pool cannot touch PSUM and has no TensorScalar/STT on this
    toolchain; it only carries the residual add (TT) on the strided h2
    view.  x lands twice: xt (conv1 taps) and xres (row-parity residual
    layout) so the late radd never gates the next xt prefetch DMA.
  - emission is software-pipelined conv1(b) x2 : conv2(b-1) x1 with
    dwt(b-2) at superblock end; x DMA prefetched a superblock ahead;
    PE gaps stay under ~3.2us -- the p-state ramp survives (gaps >3.7us
    reset the tensor clock to 1.2GHz for 3us, empirically).
  - mish chains span 4 conv1 psum groups (2048 cols) / 2 conv2 tiles
    (1024): psum release stays per-512-group via zb, but exp/square/
    MISH_POST amortize their fixed costs over wide tiles.  PE ends at
    ~94% busy; remaining idle is the ~4.6us warm-up (first weight +
    x-chunk DMA dispatch) and ~8us drain (the last block runs narrow
    chains with alternating radd engines and per-chain output DMAs).
    Only w1s and the first x chunk precede the first matmul in the
    sync-DMA queue; every other constant loads behind them.
"""
import os
import sys
from contextlib import ExitStack

sys.path.insert(0, "/opt/trn_rl_repo")

import numpy as np

_CACHE = {}

# MISH_POST custom-DVE constants: D = m + C0 (m = 0.5*(1+e^z)^2 from ACT),
# seed y0 = bitcast(~D)*C1, one NR y1 = y0*(C2 - D*y0) ~= 1/D, out =
# zb*(1 - y1) = mish(zb).  (C1, C2) are minimax over D in [1, inf) subject
# to y1(D=1) == 1 exactly, which kills the q->0 tail error (max abs err of
# mish 5.3e-3, realistic rel-fro ~1.2e-3).
_MP_C = {"s0": 0.5, "s1": -0.23795, "imm2": 2.0024409}
_MISH_POST = None


def _register_mish_post():
    """Author + register the fused mish-tail custom DVE op (8/8 stages)."""
    global _MISH_POST
    if _MISH_POST is not None:
        return _MISH_POST
    import concourse.dve_ops as dve_ops
    for op in dve_ops.OPS:
        if op.name == "MISH_POST_ANT":
            _MISH_POST = op
            return op
    from concourse.dve_spec import (AluOp, Bin, C0, C1, C2, Spec, Src0,
                                    Src1, _has_src1, lower)
    from concourse.dve_uop import DveOpSpec

    D = Src0 + C0
    nd = Bin(AluOp.BITWISE_NOT, D, D)
    y0 = nd * C1
    y1 = y0 * (C2 - D * y0)
    body = Src1 - y1 * Src1

    def _ref(in0, in1, c0, c1, c2):
        Dv = (in0 + np.float32(c0)).astype(np.float32)
        ndv = (~Dv.view(np.int32)).view(np.float32)
        y0v = ndv * np.float32(c1)
        y1v = y0v * (np.float32(c2) - Dv * y0v)
        return in1 - y1v * in1

    spec = Spec(body=body, reference=_ref)
    shas = {}
    for ver in ("v3", "v4"):
        try:
            uops = lower(spec, ver=ver)
            tmp = DveOpSpec(name="MISH_POST_ANT", opcode=1, uops=uops,
                            rd1_en=_has_src1(spec))
            shas[ver] = tmp.sha(ver)
        except Exception:
            pass
    op = dve_ops.DveOp("MISH_POST_ANT", spec, subdim=False, uops_sha=shas)
    dve_ops.OPS.append(op)
    dve_ops.CUSTOM_DVE_SPECS[op.name] = spec
    dve_ops._SUB_OPCODE_FOR_NAME[op.name] = (
        max(dve_ops._SUB_OPCODE_FOR_NAME.values()) + 1)
    assert dve_ops._SUB_OPCODE_FOR_NAME[op.name] < 0x20
    _MISH_POST = op
    return op


def _fold_params(w1, b1, g1, be1, m1, v1, w2, b2, g2, be2, m2, v2,
                 wh, bh, gh, beh, mh, vh):
    eps = 1e-5
    f64 = np.float64
    s1 = (g1.astype(f64) / np.sqrt(v1.astype(f64) + eps))
    bv1 = ((b1.astype(f64) - m1) * s1 + be1)
    w1t = (w1.astype(f64) * s1[:, None, None, None]).transpose(2, 3, 1, 0)
    w1t = np.ascontiguousarray(w1t.reshape(9, 64, 128), dtype=np.float32)
    # paired taps: (dy=-1, dy=0) stacked on K for each dx
    w1pt = np.zeros((3, 128, 128), dtype=np.float32)
    for dxi in range(3):
        w1pt[dxi, 0:64, :] = w1t[dxi]       # dy=-1
        w1pt[dxi, 64:128, :] = w1t[3 + dxi]  # dy=0

    s2 = (g2.astype(f64) / np.sqrt(v2.astype(f64) + eps))
    bv2 = ((b2.astype(f64) - m2) * s2 + be2)
    w2t = (w2.astype(f64) * s2[:, None, None, None]).transpose(2, 3, 1, 0)
    w2t = np.ascontiguousarray(w2t.reshape(9, 128, 64), dtype=np.float32)
    # conv2 M=128 blocks: blk = dxi*4 + di, di = d+1 for d in -1..2.
    # M cols 0-63  = tap (dy=d,   dx) -> output row r   (psum parts 0-63)
    # M cols 64-127= tap (dy=d-1, dx) -> output row r+1 (psum parts 64-127)
    w2p = np.zeros((12, 128, 128), dtype=np.float32)
    for dxi in range(3):
        for di in range(4):
            d = di - 1
            blk = dxi * 4 + di
            if -1 <= d <= 1:
                w2p[blk, :, 0:64] = w2t[(d + 1) * 3 + dxi]
            if -1 <= d - 1 <= 1:
                w2p[blk, :, 64:128] = w2t[d * 3 + dxi]

    sh = (gh.astype(f64) / np.sqrt(vh.astype(f64) + eps))
    bvh = ((bh.astype(f64) - mh) * sh + beh)
    whm = wh[:, :, 0, 0].astype(f64)  # [64, 192]
    wH, wV, wD = whm[:, :64], whm[:, 64:128], whm[:, 128:]
    # fused DWT + convh, K=128 = (ch x row-parity aa), M=128 = [high | cA]
    # (high band in M 0-63 so its mish chain runs at partition offset 0):
    # wdwt[bb][k=(c,aa), o<64]  = 0.5*(wH*sH + wV*sV + wD*sD)*sh  (high)
    # wdwt[bb][k=(c,aa), o>=64] = 0.5*delta(c==o)                 (cA band)
    wdwt = np.zeros((2, 128, 128), dtype=np.float32)
    for bb in (0, 1):
        for aa in (0, 1):
            sH = 1.0 if aa == 0 else -1.0
            sV = 1.0 if bb == 0 else -1.0
            sD = 1.0 if aa == bb else -1.0
            wp = 0.5 * (wH * sH + wV * sV + wD * sD) * sh[:, None]  # [o, c]
            wdwt[bb, aa * 64:(aa + 1) * 64, 0:64] = wp.T.astype(np.float32)
            wdwt[bb, aa * 64:(aa + 1) * 64, 64:128] = 0.5 * np.eye(
                64, dtype=np.float32)

    bv1 = bv1.astype(np.float32).reshape(128, 1)
    bv2d = np.tile(bv2.astype(np.float32), 2).reshape(128, 1)
    bvhd = np.tile(bvh.astype(np.float32), 2).reshape(128, 1)
    return w1t, w1pt, bv1, w2p, bv2d, wdwt, bvhd


class _Builder:
    EMAP = {"zb1": "act", "zb2": "dve", "zbD": "dve", "radd": "pool"}
    BUFS = {"ps": 5, "ps2": 3, "cw1": 4, "zb": 4, "u": 1}

    def __init__(self, H, W, finalize=True, reps=1):
        self.finalize = finalize
        self.reps = reps
        import concourse.bass as bass
        import concourse.bacc as bacc
        import concourse.mybir as mybir
        from concourse.dt import dt
        from concourse.tile import TileContext
        from concourse.alu_op_type import AluOpType

        self.bass = bass
        self.bacc = bacc
        self.mybir = mybir
        self.F32, self.F32R = dt.float32, dt.float32r
        self.Act = mybir.ActivationFunctionType
        self.Alu = AluOpType
        self.H, self.W = H, W
        self.BLOCK = 16
        self.NB = H // self.BLOCK
        self.TileContext = TileContext

    def build(self):
        H, W = self.H, self.W
        F32, F32R = self.F32, self.F32R
        HW2 = (H // 2) * (W // 2)
        nc = self.bacc.Bacc(None, target_bir_lowering=False)
        self.nc = nc
        self.pool = nc.gpsimd  # EngineType.Pool, native tensor ops

        self.params = {}
        for nm, shp, dtp in (
            ("w1t", [9, 64, 128], F32R), ("w1pt", [3, 128, 128], F32R),
            ("w2p", [12, 128, 128], F32R), ("wdwt", [2, 128, 128], F32R),
            ("bv1", [128, 1], F32), ("bv2", [128, 1], F32),
            ("bvh", [128, 1], F32),
        ):
            self.params[nm] = nc.declare_dram_parameter(nm, shp, dtp,
                                                        isOutput=False)
        # x arrives host-padded: [64, H+2 rows, W+2 cols], zero borders
        self.x = nc.declare_dram_parameter("x", [64, (H + 2) * (W + 2)], F32R,
                                           isOutput=False)
        xlo = nc.declare_dram_parameter("x_low", [64, HW2], F32, isOutput=True)
        xhi = nc.declare_dram_parameter("x_high", [64, HW2], F32,
                                        isOutput=True)
        self.xlo3 = xlo.rearrange("c (i j) -> c i j", j=W // 2)
        self.xhi3 = xhi.rearrange("c (i j) -> c i j", j=W // 2)

        with self.TileContext(nc) as tc:
            with ExitStack() as st:
                p = {}
                for name, bufs, space in (
                    ("const", 1, "SBUF"), ("xt", 2, "SBUF"),
                    ("xres", 2, "SBUF"),
                    ("h1", 2, "SBUF"), ("h2", 2, "SBUF"),
                    ("cA", 2, "SBUF"), ("xh", 2, "SBUF"),
                    ("zb", self.BUFS.get("zb", 8), "SBUF"),
                    ("u", self.BUFS.get("u", 2), "SBUF"),
                    ("g", self.BUFS.get("g", 2), "SBUF"),
                    ("ps", self.BUFS.get("ps", 4), "PSUM"),
                    ("ps2", self.BUFS.get("ps2", 4), "PSUM"),
                ):
                    p[name] = st.enter_context(
                        tc.tile_pool(name=name, bufs=bufs, space=space))
                self.p = p
                self._emit_constants()
                if self.reps == 1:
                    self._emit_pipeline()
                else:
                    with tc.For_i(0, self.reps, 1):
                        self._emit_pipeline()
        if self.finalize:
            nc.finalize()
        return nc

    def _dram(self, name):
        return self.params[name]

    def _emit_constants(self):
        # conv1's weights + bias only -- the rest loads after block 0's
        # x prefetch so PE's first matmul isn't stuck behind 1.4MB of DMA
        nc, p = self.nc, self.p
        F32, F32R = self.F32, self.F32R
        self.w1s = p["const"].tile([64, 9 * 128], F32R, tag="w1s")
        nc.sync.dma_start(
            out=self.w1s.rearrange("k (t m) -> k t m", m=128),
            in_=self._dram("w1t").rearrange("t k m -> k t m"))
        # bv1s/w1ps are not needed by block 0's first (9-singles) group:
        # they load after the first x chunk (see _emit_xt_dma b==0)
        self.bv1s = p["const"].tile([128, 1], F32, tag="bv1s")
        self.w1ps = p["const"].tile([128, 3 * 128], F32R, tag="w1ps")
        # sqrt(0.5) per-partition scalar: ACT square emits 0.5*(u+1)^2
        self.rhalf = p["const"].tile([128, 1], F32, tag="rhalf")
        nc.vector.memset(self.rhalf[:], 0.5 ** 0.5)

    def _emit_constants_late(self):
        nc, p = self.nc, self.p
        F32, F32R = self.F32, self.F32R
        self.w2s = p["const"].tile([128, 12 * 128], F32R, tag="w2s")
        nc.sync.dma_start(
            out=self.w2s.rearrange("k (t m) -> k t m", m=128),
            in_=self._dram("w2p").rearrange("t k m -> k t m"))
        self.wps = p["const"].tile([128, 2 * 128], F32R, tag="wps")
        nc.sync.dma_start(
            out=self.wps.rearrange("k (t m) -> k t m", m=128),
            in_=self._dram("wdwt").rearrange("t k m -> k t m"))
        self.bv2s = p["const"].tile([128, 1], F32, tag="bv2s")
        nc.sync.dma_start(out=self.bv2s[:], in_=self._dram("bv2")[:])
        self.bvhs = p["const"].tile([128, 1], F32, tag="bvhs")
        nc.sync.dma_start(out=self.bvhs[:], in_=self._dram("bvh")[:])

    def _emit_mms(self, mms):
        for i, (o, l, rr) in enumerate(mms):
            self.nc.tensor.matmul(o, l, rr, start=(i == 0),
                                  stop=(i == len(mms) - 1))

    def _mish_zb(self, psum_ap, bias_ap, zb_ap, eng):
        """zb = psum + bias -- releases the PSUM bank after one op; the
        rest of the mish chain runs out of SBUF.  (Pool cannot read PSUM
        on real HW, so only ACT/DVE are legal here.)"""
        if eng == "act":
            self.nc.scalar.activation(zb_ap, psum_ap, self.Act.Identity,
                                      bias=bias_ap)
        else:
            self.nc.vector.tensor_scalar_add(zb_ap, psum_ap, bias_ap)

    def _mish_tail(self, zb_ap, cols, out_ap, p0, p1, d_eng=None,
                   out_eng=None):
        """out = mish(zb) in THREE passes:

          u = exp(zb)                       (ACT)
          m = (r2*u + r2)^2 = 0.5*(u+1)^2   (ACT square, r2 = sqrt(0.5))
          out = MISH_POST(m, zb)            (custom DVE, 8 ALU stages:
                D = m+0.5; y1 = NR(bitnot-seed(D)) ~= 1/D = 2/((1+u)^2+1);
                out = zb - y1*zb = zb*tanh(softplus(zb)))
        """
        nc = self.nc
        F32, Act = self.F32, self.Act
        wmax = self.BUFS.get("cw1", 2) * 512
        u = self.p["u"].tile([128, wmax], F32, tag="u")
        g = self.p["g"].tile([128, wmax], F32, tag="g")
        u, g = u[p0:p1, 0:cols], g[p0:p1, 0:cols]
        nc.scalar.activation(u, zb_ap, Act.Exp)
        nc.scalar.activation(g, u, Act.Square, bias=self.rhalf[p0:p1],
                             scale=self.rhalf[p0:p1])
        c = _MP_C
        outs = out_ap if isinstance(out_ap, list) else [(out_ap, 0, cols)]
        for oap, c0, c1 in outs:
            nc.vector._custom_dve(
                _MISH_POST, out=oap, in0=g[:, c0:c1], in1=zb_ap[:, c0:c1],
                s0=c["s0"], s1=c["s1"], imm2=c["imm2"])

    def _conv1_group(self, a, n, psum, xdv, rx0):
        """6-matmul emission: 3 paired (dy=-1,0; K=128) + 3 single (dy=+1).

        Falls back to 9 singles when row a-1 < 0 (first group of image).
        """
        H, W = self.H, self.W
        pv = psum.rearrange("p (rr c) -> p rr c", c=W)
        mms = []
        if a >= 1:
            for dx in (0, 1, -1):
                rsl = xdv[:, a - 1 - rx0:a - 1 - rx0 + n, dx + 1:dx + 1 + W]
                mms.append((pv[:, 0:n, :],
                            self.w1ps[:, (dx + 1) * 128:(dx + 2) * 128], rsl))
            for dx in (0, 1, -1):
                rows = [rr for rr in range(a, a + n) if rr + 1 <= H - 1]
                if not rows:
                    continue
                t = 2 * 3 + (dx + 1)
                i0, nr = rows[0] - a, len(rows)
                rsl = xdv[0:64, rows[0] + 1 - rx0:rows[0] + 1 - rx0 + nr,
                          dx + 1:dx + 1 + W]
                mms.append((pv[:, i0:i0 + nr, :],
                            self.w1s[:, t * 128:(t + 1) * 128], rsl))
        else:
            for dy in (0, -1, 1):
                for dx in (0, 1, -1):
                    rows = [rr for rr in range(a, a + n)
                            if 0 <= rr + dy <= H - 1]
                    if not rows:
                        continue
                    t = (dy + 1) * 3 + (dx + 1)
                    i0, nr = rows[0] - a, len(rows)
                    rsl = xdv[0:64,
                              rows[0] + dy - rx0:rows[0] + dy - rx0 + nr,
                              dx + 1:dx + 1 + W]
                    mms.append((pv[:, i0:i0 + nr, :],
                                self.w1s[:, t * 128:(t + 1) * 128], rsl))
        self._emit_mms(mms)

    def _conv2_mms(self, ya, psl, h1v, a0):
        """M=128 row-pair conv2: 12 matmuls (3 dx x 4 d) of ap=W.

        Input = single h1 row ya+d; weight block blk=dxi*4+(d+1) has
        tap d in M 0-63 (row ya) and tap d-1 in M 64-127 (row ya+1).
        """
        H, W = self.H, self.W
        mms = []
        for dxi in range(3):
            for di in range(4):
                d = di - 1
                ri = ya + d
                if ri < 0 or ri > H - 1:
                    continue
                blk = dxi * 4 + di
                rsl = h1v[:, ri - a0, dxi:dxi + W]
                mms.append((psl[:, 0:W],
                            self.w2s[:, blk * 128:(blk + 1) * 128], rsl))
        return mms

    def _block_meta(self, b):
        H, BLOCK = self.H, self.BLOCK
        r0 = b * BLOCK
        a0 = 0 if b == 0 else r0 - 1
        a1 = min(r0 + BLOCK, H - 1)
        groups = []
        a = a0 if b == 0 else a0 + 2  # rows a0..a0+1 copied from prev block
        while a <= a1:
            n = 2 if a + 1 <= a1 else 1
            groups.append((a, n))
            a += n
        rx0 = max(a0 - 1, 0)
        rx1 = min(a1 + 1, H - 1)
        return r0, a0, groups, rx0, rx1

    def _emit_xt_dma(self, b):
        """Prefetch x rows for block b (issued one superblock ahead)."""
        nc, p = self.nc, self.p
        W, Wp = self.W, self.W + 2
        BLOCK = self.BLOCK
        r0, a0, groups, rx0, rx1 = self._block_meta(b)
        nxr = rx1 - rx0 + 1
        xt = p["xt"].tile([128, 20 * Wp], self.F32R, tag="xt")
        if b == 0:
            # block 0 has no pipeline warm-up ahead of it: split the load
            # into row chunks so the first conv1 groups start ~7us earlier
            chunks = [(c0, min(c0 + 5, nxr)) for c0 in range(0, nxr, 5)]
            # group 0's 9-singles fallback reads the lower half only --
            # load lower chunk 0 first so PE starts ~2us sooner
            order = [(0, "lo"), (0, "hi"), (1, "lo"), (1, "hi")] + [
                (i, h) for i in range(2, len(chunks)) for h in ("lo", "hi")]
            for k, (i, half) in enumerate(order):
                c0, c1 = chunks[i]
                if half == "lo":
                    nc.sync.dma_start(
                        out=xt[0:64, c0 * Wp:c1 * Wp],
                        in_=self.x[:, (rx0 + c0) * Wp:(rx0 + c1) * Wp])
                else:
                    nc.sync.dma_start(
                        out=xt[64:128, c0 * Wp:c1 * Wp],
                        in_=self.x[:, (rx0 + c0 + 1) * Wp:(rx0 + c1 + 1) * Wp])
                if k == 0:
                    nc.sync.dma_start(out=self.bv1s[:],
                                      in_=self._dram("bv1")[:])
                    nc.sync.dma_start(
                        out=self.w1ps.rearrange("k (t m) -> k t m", m=128),
                        in_=self._dram("w1pt").rearrange("t k m -> k t m"))
        else:
            nc.sync.dma_start(
                out=xt[0:64, 0:nxr * Wp],
                in_=self.x[:, rx0 * Wp:(rx1 + 1) * Wp])
            # upper half: same rows shifted by +1 (for K-paired conv1 taps)
            nc.sync.dma_start(
                out=xt[64:128, 0:(nxr - 1) * Wp],
                in_=self.x[:, (rx0 + 1) * Wp:(rx1 + 1) * Wp])
        # residual rows in a dedicated tile [128p = ch x row-parity,
        # 8 gi x 256 w] so xt is freed by the conv1 matmuls alone (the
        # late residual add must not gate the next xt prefetch DMA)
        xres = p["xres"].tile([128, (BLOCK // 2) * W], self.F32R,
                              tag="xres")
        xrv = xres.rearrange("p (gi w) -> p gi w", w=W)
        x3 = self.x.rearrange("c (r w) -> c r w", w=Wp)
        nc.sync.dma_start(out=xrv[0:64, :, :],
                          in_=x3[:, r0:r0 + BLOCK:2, 1:W + 1])
        nc.sync.dma_start(out=xrv[64:128, :, :],
                          in_=x3[:, r0 + 1:r0 + BLOCK:2, 1:W + 1])
        self._st[b] = {"a0": a0, "rx0": rx0, "r0": r0, "xt": xt,
                       "xres": xres, "groups": groups}

    def _conv1_units(self, b):
        """Return per-group closures (6 matmuls + mish) -> h1(b)."""
        nc, p = self.nc, self.p
        W = self.W
        F32, F32R = self.F32, self.F32R
        st = self._st[b]
        a0, rx0, groups, xt = st["a0"], st["rx0"], st["groups"], st["xt"]
        Wp = W + 2
        xdv = xt.rearrange("p (rr c) -> p rr c", c=Wp)

        h1 = p["h1"].tile([128, 18 * Wp], F32R, tag="h1")
        h1v = h1.rearrange("p (rr c) -> p rr c", c=Wp)
        h1vf = h1.bitcast(F32).rearrange("p (rr c) -> p rr c", c=Wp)
        nc.vector.memset(h1vf[:, 0:18, 0:1], 0.0)
        nc.vector.memset(h1vf[:, 0:18, W + 1:W + 2], 0.0)
        if b > 0:
            # halo reuse: rows a0..a0+1 already computed by the previous
            # block -- SBUF-to-SBUF DMA copy instead of recomputing their
            # 6 matmuls + mish (keeps the compute engines free)
            prev = self._st[b - 1]
            si = a0 - prev["a0"]
            nc.sync.dma_start(out=h1v[:, 0:2, :],
                              in_=prev["h1v"][:, si:si + 2, :])
        st["h1v"] = h1v

        state = {}
        cw1 = self.BUFS.get("cw1", 2)  # conv1 groups per mish chain

        def unit(k, ga, gn, last):
            def run():
                cols = gn * W
                psum = p["ps"].tile([128, 2 * W], F32, tag="ps")
                self._conv1_group(ga, gn, psum[:, 0:cols], xdv, rx0)
                if k % cw1 == 0:
                    state["zb"] = self.p["zb"].tile([128, cw1 * 512], F32,
                                                    name="zbt", tag="zb")
                    state["ga0"] = ga
                    state["cols"] = 0
                zb = state["zb"]
                off = state["cols"]
                self._mish_zb(psum[:, 0:cols], self.bv1s[:],
                              zb[:, off:off + cols], self.EMAP["zb1"])
                state["cols"] = off + cols
                if last or k % cw1 == cw1 - 1:
                    tcols = state["cols"]
                    lr0 = state["ga0"] - a0
                    nrow = tcols // W
                    self._mish_tail(zb[:, 0:tcols], tcols,
                                    h1v[:, lr0:lr0 + nrow, 1:W + 1],
                                    0, 128)
            return run
        return [unit(k, ga, gn, k == len(groups) - 1)
                for k, (ga, gn) in enumerate(groups)]

    def _conv2_units(self, b):
        """Return per-2-gi-tile closures (24 matmuls + mish + residual)."""
        nc, p = self.nc, self.p
        W, BLOCK, Wh = self.W, self.BLOCK, self.W // 2
        F32, F32R = self.F32, self.F32R
        st = self._st[b]
        r0, a0, rx0 = st["r0"], st["a0"], st["rx0"]
        h1v, xt = st["h1v"], st["xt"]

        # h2 layout: [128p = ch x row-parity, 8 gi x 2 bb x 128 j]
        h2 = p["h2"].tile([128, (BLOCK // 2) * W], F32R, tag="h2")
        h2v = h2.rearrange("p (gi bb j) -> p gi j bb", bb=2, j=Wh)
        xrv = st["xres"].bitcast(F32).rearrange("p (gi w) -> p gi w", w=W)
        st["h2"] = h2

        state = {}
        # chain width: cw2 conv2 tiles (2 gi each) share one mish chain;
        # the last block runs narrow (1-tile chains) to shorten the drain
        cw2 = 1 if b == self.NB - 1 else self.BUFS.get("cw2", 2)

        def unit(g2):
            def run():
                psf = p["ps2"].tile([128, 2 * W], F32, tag="ps2")
                for gi in (2 * g2, 2 * g2 + 1):
                    ya = r0 + gi * 2
                    self._emit_mms(self._conv2_mms(
                        ya, psf[:, (gi - 2 * g2) * W:(gi - 2 * g2 + 1) * W],
                        h1v, a0))
                if g2 % cw2 == 0:
                    state["zb"] = self.p["zb"].tile(
                        [128, max(cw2, 2) * 512], F32, name="zbt", tag="zb")
                    state["g0"] = g2
                off = (g2 % cw2) * 2 * W
                self._mish_zb(psf[:, 0:2 * W], self.bv2s[:],
                              state["zb"][:, off:off + 2 * W],
                              self.EMAP["zb2"])
                if g2 % cw2 == cw2 - 1:
                    # mish straight into the (j, bb)-strided h2 view, then
                    # residual add (xres rows are gi-aligned row pairs)
                    g4 = state["g0"]
                    ngi = 2 * cw2
                    outs = [(h2v[:, 2 * g4 + i, :, :], i * W, (i + 1) * W)
                            for i in range(ngi)]
                    self._mish_tail(state["zb"][:, 0:ngi * W], ngi * W,
                                    outs, 0, 128)
                    r_eng = self.EMAP.get("radd", "pool")
                    if cw2 == 1:
                        r_eng = "pool" if g2 % 2 == 0 else "dve"
                    radd_e = (self.pool if r_eng == "pool"
                              else nc.vector)
                    radd_e.tensor_add(
                        out=h2v[:, 2 * g4:2 * g4 + ngi, :, :],
                        in0=h2v[:, 2 * g4:2 * g4 + ngi, :, :],
                        in1=xrv[:, 2 * g4:2 * g4 + ngi, :])
            return run
        return [unit(g2) for g2 in range(BLOCK // 4)]

    def _emit_dwt(self, b):
        """Fused DWT + convh: 2 matmuls per hg, M=128 = [cA | high]."""
        nc, p = self.nc, self.p
        W, BLOCK, Wh = self.W, self.BLOCK, self.W // 2
        F32 = self.F32
        st = self._st[b]
        h2 = st["h2"]
        NP = BLOCK // 2
        h2r = h2.rearrange("p (gi bb j) -> p gi bb j", bb=2, j=Wh)
        # cA rides psum parts 64-127; high band (mish) parts 0-63
        cat = p["cA"].tile([128, NP * Wh], F32, tag="cA")
        xht = p["xh"].tile([64, NP * Wh], F32, tag="xh")
        zb = p["zb"].tile([128, 1024], F32, tag="zb")
        narrow = (b == self.NB - 1)
        for h2g in range(NP // 4):
            psf = p["ps2"].tile([128, 2 * W], F32, tag="ps2")
            for hgi in (0, 1):
                hg = 2 * h2g + hgi
                mms = []
                for bb in (0, 1):
                    rsl = h2r[:, 2 * hg:2 * hg + 2, bb, :]
                    mms.append((psf[:, hgi * W:(hgi + 1) * W],
                                self.wps[:, bb * 128:(bb + 1) * 128], rsl))
                self._emit_mms(mms)
            # cA band copy (DMA cannot read PSUM; Pool cannot either)
            self.nc.scalar.activation(
                cat[64:128, 2 * h2g * W:(2 * h2g + 2) * W],
                psf[64:128, 0:2 * W], self.Act.Identity)
            self._mish_zb(psf[0:64, 0:2 * W], self.bvhs[0:64],
                          zb[0:64, 2 * h2g * W:(2 * h2g + 2) * W],
                          self.EMAP["zbD"])
            if narrow:
                self._mish_tail(zb[0:64, 2 * h2g * W:(2 * h2g + 2) * W],
                                2 * W,
                                xht[0:64, 2 * h2g * W:(2 * h2g + 2) * W],
                                0, 64)
                r4 = NP * b + 4 * h2g
                nc.sync.dma_start(
                    out=self.xlo3[:, r4:r4 + 4, :],
                    in_=cat[64:128, 2 * h2g * W:(2 * h2g + 2) * W]
                    .rearrange("c (pr j) -> c pr j", j=Wh))
                nc.sync.dma_start(
                    out=self.xhi3[:, r4:r4 + 4, :],
                    in_=xht[0:64, 2 * h2g * W:(2 * h2g + 2) * W]
                    .rearrange("c (pr j) -> c pr j", j=Wh))
        if not narrow:
            self._mish_tail(zb[0:64, 0:4 * W], 4 * W, xht[0:64, 0:4 * W],
                            0, 64)
            nc.sync.dma_start(
                out=self.xlo3[:, NP * b:NP * (b + 1), :],
                in_=cat[64:128, :].rearrange("c (pr j) -> c pr j", j=Wh))
            nc.sync.dma_start(
                out=self.xhi3[:, NP * b:NP * (b + 1), :],
                in_=xht[0:64, :].rearrange("c (pr j) -> c pr j", j=Wh))
        del self._st[b]

    def _emit_pipeline(self):
        """Software-pipelined emission: dwt(b-2) | conv1(b) x conv2(b-1).

        The PE queue is in-order, so each stage only reads tiles whose
        producer chains ran >=1 full stage earlier, and conv1/conv2 tiles
        interleave (2:1) so every psum pool's reuse distance spans ~15us
        of queued PE work -- PE stays busy and at full p-state."""
        self._st = {}
        NB = self.NB
        for b in range(NB + 1):
            if b == 0:
                self._emit_xt_dma(0)
                self._emit_constants_late()
            if b + 1 < NB:
                self._emit_xt_dma(b + 1)  # prefetch one superblock ahead
            A = self._conv1_units(b) if b < NB else []
            B = self._conv2_units(b - 1) if 1 <= b <= NB else []
            ia = ib = 0
            ratio = self.BUFS.get("ratio", 2)
            while ia < len(A) or ib < len(B):
                for _ in range(ratio):
                    if ia < len(A):
                        A[ia]()
                        ia += 1
                if ib < len(B):
                    B[ib]()
                    ib += 1
            # dwt(b-2) at superblock end: h2(b-2)'s final residual add
            # (end of a deep chain) completes early in THIS superblock,
            # so by now PE can read h2 without stalling.  The last dwt
            # rides one superblock early (right after its narrow conv2
            # chains) to shorten the pipeline drain.
            if b >= 2:
                self._emit_dwt(b - 2)
            if b == NB and (NB - 1) in self._st:
                self._emit_dwt(NB - 1)


def _build(H, W, finalize=True, reps=1):
    _register_mish_post()
    return _Builder(H, W, finalize=finalize, reps=reps).build()


def _get_program(H, W):
    key = (H, W)
    if key not in _CACHE:
        _CACHE[key] = _build(H, W)
    return _CACHE[key]


def kernel(x, w1, b1, g1, be1, m1, v1, w2, b2, g2, be2, m2, v2,
           wh, bh, gh, beh, mh, vh):
    from concourse.bass_utils import run_bass_kernel_spmd

    x = np.asarray(x, dtype=np.float32)
    B, C, H, W = x.shape
    w1t, w1pt, bv1, w2p, bv2d, wdwt, bvhd = _fold_params(
        np.asarray(w1, np.float32), np.asarray(b1, np.float32),
        np.asarray(g1, np.float32), np.asarray(be1, np.float32),
        np.asarray(m1, np.float32), np.asarray(v1, np.float32),
        np.asarray(w2, np.float32), np.asarray(b2, np.float32),
        np.asarray(g2, np.float32), np.asarray(be2, np.float32),
        np.asarray(m2, np.float32), np.asarray(v2, np.float32),
        np.asarray(wh, np.float32), np.asarray(bh, np.float32),
        np.asarray(gh, np.float32), np.asarray(beh, np.float32),
        np.asarray(mh, np.float32), np.asarray(vh, np.float32))

    nc = _get_program(H, W)
    core_ids = list(range(B))
    xp = np.zeros((B, C, H + 2, W + 2), dtype=np.float32)
    xp[:, :, 0:H, 1:W + 1] = x
    in_maps = []
    for i in range(B):
        in_maps.append({
            "x": np.ascontiguousarray(xp[i].reshape(C, (H + 2) * (W + 2))),
            "w1t": w1t, "w1pt": w1pt, "w2p": w2p, "wdwt": wdwt,
            "bv1": bv1, "bv2": bv2d, "bvh": bvhd,
        })
    trace = os.environ.get("KERNEL_TRACE", "0") == "1"
    try:
        res = run_bass_kernel_spmd(nc, in_maps, core_ids, trace=trace)
    except ModuleNotFoundError:
        res = run_bass_kernel_spmd(nc, in_maps, core_ids, trace=False)
    if res.exec_time_ns is not None:
        print(f"HW exec time: {res.exec_time_ns} ns")
    H2, W2 = H // 2, W // 2
    x_low = np.stack([res.results[i]["x_low"].reshape(C, H2, W2)
                      for i in range(B)])
    x_high = np.stack([res.results[i]["x_high"].reshape(C, H2, W2)
                       for i in range(B)])
    return (x_low, x_high)
